# revision 12
# baseline (speedup 1.0000x reference)
"""Trainium2 Bass kernel for nn_KKLayer (spectral channel-mix layer).

Math identity: the reference computes
    y = Re(IFFT2((A + iB) . conj(FFT2(x))))            (channel mix in freq domain)
Since channel mixing commutes with the spatial FFT and, for real x,
IFFT2(conj(FFT2(x))) is x spatially "negated" (h -> (-h) mod H, w -> (-w) mod W),
the whole layer collapses to
    y[b,o,h,w] = sum_i A[o,i] * x[b,i,(H-h)%H,(W-w)%W]
(betas drop out of the real part entirely).

Kernel: data-parallel over batch (8 batches -> 8 cores). The spatial flip is a
pure layout permutation, applied on the host while converting x to fp16, so
every device access is contiguous. Device I/O is fp16 (rel-err gate is 2e-2;
measured ~5e-4), halving HBM traffic vs fp32: 4MB in + 4MB out per core
(~23.5us at the 358 GB/s per-core HBM cap) instead of 16MB (~47us).

Per core:
  - load alphas^T (fp16 stationary weights) + x[b] (fp16, pre-flipped) in 8
    contiguous 512KB chunks
  - per chunk: 4 matmuls [K=128, M=128, N=512] fp16 -> fp32 PSUM
  - PSUM->SBUF copies cast fp32 -> fp16 (2 on DVE + 2 on ACT per chunk)
  - contiguous 512KB fp16 DMA-out per chunk
"""

import numpy as np

import concourse.bass as bass
import concourse.bacc as bacc
import concourse.mybir as mybir
from concourse import tile
from concourse.bass_utils import run_bass_kernel_spmd

B, CIN, COUT, H, W = 8, 128, 128, 128, 128
HW = H * W              # 16384
N_CORES = 8
CHUNK = 4096            # columns per DMA chunk (8KB/partition in fp16)
NCH = HW // CHUNK       # 4 chunks
MMN = 512               # matmul free dim (one PSUM bank of fp32)
MM_PER_CH = CHUNK // MMN

F16 = mybir.dt.float16
F32 = mybir.dt.float32

# dest row h reads source row (H - h) % H
_FLIP = (-np.arange(H)) % H


def _build_nc():
    nc = bacc.Bacc(None, target_bir_lowering=False)
    # chunk-major layout: chunk k is a contiguous 1MB block in DRAM
    x = nc.dram_tensor("x", [NCH, CIN, CHUNK], F16, kind="ExternalInput")
    wT = nc.dram_tensor("wT", [CIN, COUT], F16, kind="ExternalInput")
    y = nc.dram_tensor("y", [NCH, COUT, CHUNK], F16, kind="ExternalOutput")

    with tile.TileContext(nc) as tc:
        with (
            tc.tile_pool(name="wp", bufs=1) as wpool,
            tc.tile_pool(name="xp", bufs=1) as xpool,
            tc.tile_pool(name="yp", bufs=1) as ypool,
            tc.tile_pool(name="ps", bufs=8, space="PSUM") as pspool,
        ):
            # HWDGE writes descriptor rings in partition order, so the
            # engine serving partitions 92-95/124-127 is otherwise fed
            # ~2us late on every DMA; issue that partition range first
            def dma2(dst, src):
                nc.sync.dma_start(dst[92:128], src[92:128])
                nc.sync.dma_start(dst[0:92], src[0:92])

            w_t = wpool.tile([CIN, COUT], F16)
            dma2(w_t, wT)

            # all input DMAs issued first: no waits, so they stream
            # back-to-back on the sync HWDGE ring
            xch = []
            for k in range(NCH):
                t = xpool.tile([CIN, CHUNK], F16, tag=f"x{k}", name=f"xch{k}")
                dma2(t, x[k])
                xch.append(t)

            ych = [
                ypool.tile([COUT, CHUNK], F16, tag=f"y{k}", name=f"ych{k}")
                for k in range(NCH)
            ]

            for k in range(NCH):
                for j in range(MM_PER_CH):
                    ps = pspool.tile(
                        [COUT, MMN], F32, tag="ps", name=f"ps{k}_{j}"
                    )
                    nc.tensor.matmul(
                        ps[:],
                        w_t[:],
                        xch[k][:, MMN * j:MMN * (j + 1)],
                        start=True,
                        stop=True,
                    )
                    dst = ych[k][:, MMN * j:MMN * (j + 1)]
                    # split evacuation DVE/ACT so neither lags the DMA cadence
                    if j % 2 == 0:
                        nc.vector.tensor_copy(dst, ps[:])
                    else:
                        nc.scalar.copy(dst, ps[:])
                dma2(y[k], ych[k])
    nc.compile()
    return nc


_NC_CACHE = {}


def _get_nc():
    if "nc" not in _NC_CACHE:
        _NC_CACHE["nc"] = _build_nc()
    return _NC_CACHE["nc"]


def _prep_in_maps(x, alphas):
    """host prep: spatial flip + fp16 cast (layout/precision only, no math)"""
    x16 = np.asarray(x).astype(np.float16)
    xf = x16[:, :, _FLIP[:, None], _FLIP[None, :]]  # [B, CIN, H, W]
    wT = np.ascontiguousarray(
        np.asarray(alphas, dtype=np.float32).T
    ).astype(np.float16)
    return [
        {
            "x": np.ascontiguousarray(
                xf[c].reshape(CIN, NCH, CHUNK).transpose(1, 0, 2)
            ),
            "wT": wT,
        }
        for c in range(N_CORES)
    ]


def kernel(x, alphas, betas=None, **_unused):
    nc = _get_nc()
    in_maps = _prep_in_maps(x, alphas)
    res = run_bass_kernel_spmd(nc, in_maps, core_ids=list(range(N_CORES)))
    out = np.stack(
        [
            np.asarray(res.results[c]["y"], dtype=np.float32)
            .transpose(1, 0, 2)
            .reshape(COUT, H, W)
            for c in range(N_CORES)
        ]
    )
    return out


# revision 13
# speedup vs baseline: 1.8965x; 1.8965x over previous
"""Trainium2 Bass kernel for nn_KKLayer (spectral channel-mix layer).

Math identity: the reference computes
    y = Re(IFFT2((A + iB) . conj(FFT2(x))))            (channel mix in freq domain)
Since channel mixing commutes with the spatial FFT and, for real x,
IFFT2(conj(FFT2(x))) is x spatially "negated" (h -> (-h) mod H, w -> (-w) mod W),
the whole layer collapses to
    y[b,o,h,w] = sum_i A[o,i] * x[b,i,(H-h)%H,(W-w)%W]
(betas drop out of the real part entirely).

Kernel: data-parallel over batch (8 batches -> 8 cores). The spatial flip is a
pure layout permutation, applied on the host while converting x to fp16, so
every device access is contiguous. Device I/O is fp16 (rel-err gate is 2e-2;
measured ~5e-4), halving HBM traffic vs fp32: 4MB in + 4MB out per core
(~23.5us at the 358 GB/s per-core HBM cap) instead of 16MB (~47us).

Per core:
  - load alphas^T (fp16 stationary weights) + x[b] (fp16, pre-flipped) in 8
    contiguous 512KB chunks
  - per chunk: 4 matmuls [K=128, M=128, N=512] fp16 -> fp32 PSUM
  - PSUM->SBUF copies cast fp32 -> fp16 (2 on DVE + 2 on ACT per chunk)
  - contiguous 512KB fp16 DMA-out per chunk
"""

import numpy as np

import concourse.bass as bass
import concourse.bacc as bacc
import concourse.mybir as mybir
from concourse import tile
from concourse.bass_utils import run_bass_kernel_spmd

B, CIN, COUT, H, W = 8, 128, 128, 128, 128
HW = H * W              # 16384
N_CORES = 8
CHUNK = 4096            # columns per DMA chunk (8KB/partition in fp16)
NCH = HW // CHUNK       # 4 chunks
MMN = 512               # matmul free dim (one PSUM bank of fp32)
MM_PER_CH = CHUNK // MMN

F16 = mybir.dt.float16
F32 = mybir.dt.float32

# dest row h reads source row (H - h) % H
_FLIP = (-np.arange(H)) % H


def _build_nc():
    nc = bacc.Bacc(None, target_bir_lowering=False)
    # chunk-major layout: chunk k is a contiguous 1MB block in DRAM
    x = nc.dram_tensor("x", [NCH, CIN, CHUNK], F16, kind="ExternalInput")
    wT = nc.dram_tensor("wT", [CIN, COUT], F16, kind="ExternalInput")
    y = nc.dram_tensor("y", [NCH, COUT, CHUNK], F16, kind="ExternalOutput")

    with tile.TileContext(nc) as tc:
        with (
            tc.tile_pool(name="wp", bufs=1) as wpool,
            tc.tile_pool(name="xp", bufs=1) as xpool,
            tc.tile_pool(name="yp", bufs=1) as ypool,
            tc.tile_pool(name="ps", bufs=8, space="PSUM") as pspool,
        ):
            # HWDGE writes descriptor rings in partition order, so the
            # engine serving partitions 92-95/124-127 is otherwise fed
            # ~2us late on every DMA. Partitions >=64 map to the odd
            # engines and <64 to the even ones (port swizzle), so a
            # [64:128]+[0:64] split keeps per-engine load balanced while
            # feeding the late engine first.
            def dma2(dst, src):
                nc.sync.dma_start(dst[64:128], src[64:128])
                nc.sync.dma_start(dst[0:64], src[0:64])

            w_t = wpool.tile([CIN, COUT], F16)
            dma2(w_t, wT)

            # all input DMAs issued first: no waits, so they stream
            # back-to-back on the sync HWDGE ring
            xch = []
            for k in range(NCH):
                t = xpool.tile([CIN, CHUNK], F16, tag=f"x{k}", name=f"xch{k}")
                dma2(t, x[k])
                xch.append(t)

            ych = [
                ypool.tile([COUT, CHUNK], F16, tag=f"y{k}", name=f"ych{k}")
                for k in range(NCH)
            ]

            for k in range(NCH):
                for j in range(MM_PER_CH):
                    ps = pspool.tile(
                        [COUT, MMN], F32, tag="ps", name=f"ps{k}_{j}"
                    )
                    nc.tensor.matmul(
                        ps[:],
                        w_t[:],
                        xch[k][:, MMN * j:MMN * (j + 1)],
                        start=True,
                        stop=True,
                    )
                    dst = ych[k][:, MMN * j:MMN * (j + 1)]
                    # split evacuation DVE/ACT so neither lags the DMA cadence
                    if j % 2 == 0:
                        nc.vector.tensor_copy(dst, ps[:])
                    else:
                        nc.scalar.copy(dst, ps[:])
                dma2(y[k], ych[k])
    nc.compile()
    return nc


_NC_CACHE = {}


def _get_nc():
    if "nc" not in _NC_CACHE:
        _NC_CACHE["nc"] = _build_nc()
    return _NC_CACHE["nc"]


def _prep_in_maps(x, alphas):
    """host prep: spatial flip + fp16 cast (layout/precision only, no math)"""
    x16 = np.asarray(x).astype(np.float16)
    xf = x16[:, :, _FLIP[:, None], _FLIP[None, :]]  # [B, CIN, H, W]
    wT = np.ascontiguousarray(
        np.asarray(alphas, dtype=np.float32).T
    ).astype(np.float16)
    return [
        {
            "x": np.ascontiguousarray(
                xf[c].reshape(CIN, NCH, CHUNK).transpose(1, 0, 2)
            ),
            "wT": wT,
        }
        for c in range(N_CORES)
    ]


def kernel(x, alphas, betas=None, **_unused):
    nc = _get_nc()
    in_maps = _prep_in_maps(x, alphas)
    res = run_bass_kernel_spmd(nc, in_maps, core_ids=list(range(N_CORES)))
    out = np.stack(
        [
            np.asarray(res.results[c]["y"], dtype=np.float32)
            .transpose(1, 0, 2)
            .reshape(COUT, H, W)
            for c in range(N_CORES)
        ]
    )
    return out


# revision 17
# speedup vs baseline: 2.3067x; 1.2163x over previous
"""Trainium2 Bass kernel for nn_KKLayer (spectral channel-mix layer).

Math identity: the reference computes
    y = Re(IFFT2((A + iB) . conj(FFT2(x))))            (channel mix in freq domain)
Since channel mixing commutes with the spatial FFT and, for real x,
IFFT2(conj(FFT2(x))) is x spatially "negated" (h -> (-h) mod H, w -> (-w) mod W),
the whole layer collapses to
    y[b,o,h,w] = sum_i A[o,i] * x[b,i,(H-h)%H,(W-w)%W]
(betas drop out of the real part entirely).

Kernel: the work is split into 64 independent units (batch x column-chunk,
each 512KB in + 512KB out in fp16) spread over the 8 cores. The spatial flip
is a pure layout permutation, applied on the host while converting x to fp16
(rel-err gate is 2e-2; measured ~5e-4), so every device access is contiguous
and HBM traffic is halved vs fp32.

Core 0 demonstrably runs its DMA engines slower than the other cores (one
engine serves host/runtime traffic and lags ~20%), so the unit counts are
load-balanced: core 0 takes 6 units, the fastest-measured cores take 9, the
rest 8 - equalizing per-core finish times. The per-core count is passed as a
scalar input and the tail unit DMAs are predicated on it (cond=), so a single
SPMD program serves all cores.

Per unit: 1 input DMA, 4 matmuls [K=128, M=128, N=512] fp16 -> fp32 PSUM,
4 PSUM->SBUF copies casting to fp16 (2 on DVE + 2 on ACT), 1 output DMA.
"""

import numpy as np

import concourse.bass as bass
import concourse.bacc as bacc
import concourse.mybir as mybir
from concourse import tile
from concourse.bass_utils import run_bass_kernel_spmd

B, CIN, COUT, H, W = 8, 128, 128, 128, 128
HW = H * W              # 16384
N_CORES = 8
UNIT = 2048             # columns per unit (4KB/partition in fp16)
KPB = HW // UNIT        # 8 column chunks per batch
N_UNITS = B * KPB       # 64 total units
MMN = 512               # matmul free dim (one PSUM bank of fp32)
MM_PER_U = UNIT // MMN  # 4

# units per core: core 0 is the host-facing (slow-DMA) core; cores 1,3
# measured fastest. sum == N_UNITS.
COUNTS = (6, 9, 8, 9, 8, 8, 8, 8)
assert sum(COUNTS) == N_UNITS
U_MAX = max(COUNTS)     # SPMD program sized for the largest share
U_MIN = min(COUNTS)     # units below this run unconditionally
STARTS = tuple(int(s) for s in np.cumsum((0,) + COUNTS[:-1]))

F16 = mybir.dt.float16
F32 = mybir.dt.float32
I32 = mybir.dt.int32

# dest row h reads source row (H - h) % H
_FLIP = (-np.arange(H)) % H


def _build_nc():
    nc = bacc.Bacc(None, target_bir_lowering=False)
    x = nc.dram_tensor("x", [U_MAX, CIN, UNIT], F16, kind="ExternalInput")
    wT = nc.dram_tensor("wT", [CIN, COUT], F16, kind="ExternalInput")
    cnt = nc.dram_tensor("cnt", [1, 1], I32, kind="ExternalInput")
    y = nc.dram_tensor("y", [U_MAX, COUT, UNIT], F16, kind="ExternalOutput")

    with tile.TileContext(nc) as tc, nc.sync.register() as cnt_reg:
        with (
            tc.tile_pool(name="cp", bufs=1) as cpool,
            tc.tile_pool(name="wp", bufs=1) as wpool,
            tc.tile_pool(name="xp", bufs=1) as xpool,
            tc.tile_pool(name="yp", bufs=1) as ypool,
            tc.tile_pool(name="ps", bufs=8, space="PSUM") as pspool,
        ):
            cnt_t = cpool.tile([1, 1], I32)
            nc.sync.dma_start(cnt_t[:], cnt[:])
            w_t = wpool.tile([CIN, COUT], F16)
            nc.sync.dma_start(w_t[:], wT[:])

            # unconditional input DMAs first: no waits, so they stream
            # back-to-back on the sync HWDGE ring
            xch = []
            for u in range(U_MIN):
                t = xpool.tile([CIN, UNIT], F16, tag=f"x{u}", name=f"xch{u}")
                nc.sync.dma_start(t[:], x[u])
                xch.append(t)

            # per-core unit count -> predicate for the tail units
            nc.sync.reg_load(cnt_reg, cnt_t[0:1, 0:1])
            cnt_v = nc.sync.snap(cnt_reg, min_val=U_MIN, max_val=U_MAX)
            conds = {u: cnt_v > u for u in range(U_MIN, U_MAX)}

            for u in range(U_MIN, U_MAX):
                t = xpool.tile([CIN, UNIT], F16, tag=f"x{u}", name=f"xch{u}")
                nc.sync.dma_start(t[:], x[u], cond=conds[u], cond_hint=True)
                xch.append(t)

            ych = [
                ypool.tile([COUT, UNIT], F16, tag=f"y{u}", name=f"ych{u}")
                for u in range(U_MAX)
            ]

            for u in range(U_MAX):
                for j in range(MM_PER_U):
                    ps = pspool.tile(
                        [COUT, MMN], F32, tag="ps", name=f"ps{u}_{j}"
                    )
                    # skipped units just matmul stale SBUF; their output
                    # DMA is predicated off so nothing escapes
                    nc.tensor.matmul(
                        ps[:],
                        w_t[:],
                        xch[u][:, MMN * j:MMN * (j + 1)],
                        start=True,
                        stop=True,
                    )
                    dst = ych[u][:, MMN * j:MMN * (j + 1)]
                    # split evacuation DVE/ACT so neither lags the DMA cadence
                    if j % 2 == 0:
                        nc.vector.tensor_copy(dst, ps[:])
                    else:
                        nc.scalar.copy(dst, ps[:])
                if u < U_MIN:
                    nc.sync.dma_start(y[u], ych[u][:])
                else:
                    nc.sync.dma_start(
                        y[u], ych[u][:], cond=conds[u], cond_hint=True
                    )
    nc.compile()
    return nc


_NC_CACHE = {}


def _get_nc():
    if "nc" not in _NC_CACHE:
        _NC_CACHE["nc"] = _build_nc()
    return _NC_CACHE["nc"]


def _prep_in_maps(x, alphas):
    """host prep: spatial flip + fp16 cast (layout/precision only, no math)
    then carve the 64 (batch, chunk) units into per-core contiguous spans"""
    x16 = np.asarray(x).astype(np.float16)
    xf = x16[:, :, _FLIP[:, None], _FLIP[None, :]].reshape(B, CIN, HW)
    wT = np.ascontiguousarray(
        np.asarray(alphas, dtype=np.float32).T
    ).astype(np.float16)
    in_maps = []
    for c in range(N_CORES):
        xarr = np.zeros((U_MAX, CIN, UNIT), dtype=np.float16)
        for u in range(COUNTS[c]):
            g = STARTS[c] + u
            b, k = divmod(g, KPB)
            xarr[u] = xf[b, :, UNIT * k:UNIT * (k + 1)]
        in_maps.append(
            {
                "x": xarr,
                "wT": wT,
                "cnt": np.array([[COUNTS[c]]], dtype=np.int32),
            }
        )
    return in_maps


def kernel(x, alphas, betas=None, **_unused):
    nc = _get_nc()
    in_maps = _prep_in_maps(x, alphas)
    res = run_bass_kernel_spmd(nc, in_maps, core_ids=list(range(N_CORES)))
    out = np.empty((B, COUT, HW), dtype=np.float32)
    for c in range(N_CORES):
        yarr = np.asarray(res.results[c]["y"], dtype=np.float32)
        for u in range(COUNTS[c]):
            g = STARTS[c] + u
            b, k = divmod(g, KPB)
            out[b, :, UNIT * k:UNIT * (k + 1)] = yarr[u]
    return out.reshape(B, COUT, H, W)


# revision 20
# speedup vs baseline: 2.3471x; 1.0175x over previous
"""Trainium2 Bass kernel for nn_KKLayer (spectral channel-mix layer).

Math identity: the reference computes
    y = Re(IFFT2((A + iB) . conj(FFT2(x))))            (channel mix in freq domain)
Since channel mixing commutes with the spatial FFT and, for real x,
IFFT2(conj(FFT2(x))) is x spatially "negated" (h -> (-h) mod H, w -> (-w) mod W),
the whole layer collapses to
    y[b,o,h,w] = sum_i A[o,i] * x[b,i,(H-h)%H,(W-w)%W]
(betas drop out of the real part entirely).

Kernel: the work is split into 64 independent units (batch x column-chunk,
each 512KB in + 512KB out in fp16) spread over the 8 cores. The spatial flip
is a pure layout permutation, applied on the host while converting x to fp16
(rel-err gate is 2e-2; measured ~5e-4), so every device access is contiguous
and HBM traffic is halved vs fp32.

Core 0 demonstrably runs its DMA engines slower than the other cores (one
engine serves host/runtime traffic and lags ~20%), so the unit counts are
load-balanced: core 0 takes 6 units, the fastest-measured cores take 9, the
rest 8 - equalizing per-core finish times. The per-core count is passed as a
scalar input and the tail unit DMAs are predicated on it (cond=), so a single
SPMD program serves all cores.

Per unit: 1 input DMA, 4 matmuls [K=128, M=128, N=512] fp16 -> fp32 PSUM,
4 PSUM->SBUF copies casting to fp16 (2 on DVE + 2 on ACT), 1 output DMA.
"""

import numpy as np

import concourse.bass as bass
import concourse.bacc as bacc
import concourse.mybir as mybir
from concourse import tile
from concourse.bass_utils import run_bass_kernel_spmd

B, CIN, COUT, H, W = 8, 128, 128, 128, 128
HW = H * W              # 16384
N_CORES = 8
UNIT = 2048             # columns per unit (4KB/partition in fp16)
KPB = HW // UNIT        # 8 column chunks per batch
N_UNITS = B * KPB       # 64 total units
MMN = 512               # matmul free dim (one PSUM bank of fp32)
MM_PER_U = UNIT // MMN  # 4

# units per core: core 0 is the host-facing (slow-DMA) core; cores 1,3
# measured fastest. sum == N_UNITS.
COUNTS = (6, 9, 8, 9, 8, 8, 8, 8)
assert sum(COUNTS) == N_UNITS
U_MAX = max(COUNTS)     # SPMD program sized for the largest share
U_MIN = min(COUNTS)     # units below this run unconditionally
STARTS = tuple(int(s) for s in np.cumsum((0,) + COUNTS[:-1]))

F16 = mybir.dt.float16
F32 = mybir.dt.float32
I32 = mybir.dt.int32

# dest row h reads source row (H - h) % H
_FLIP = (-np.arange(H)) % H


def _build_nc():
    nc = bacc.Bacc(None, target_bir_lowering=False)
    x = nc.dram_tensor("x", [U_MAX, CIN, UNIT], F16, kind="ExternalInput")
    wT = nc.dram_tensor("wT", [CIN, COUT], F16, kind="ExternalInput")
    cnt = nc.dram_tensor("cnt", [1, 1], I32, kind="ExternalInput")
    y = nc.dram_tensor("y", [U_MAX, COUT, UNIT], F16, kind="ExternalOutput")

    # units [U_MAX-cnt, U_MAX) are active on a core; units below that are
    # skipped. The unconditional units (>= N_COND) are issued/computed
    # FIRST so the conditional (possibly garbage) tail never delays a
    # core's real work, and every core's compute follows arrival order.
    N_COND = U_MAX - U_MIN
    u_order = list(range(N_COND, U_MAX)) + list(range(N_COND))

    with tile.TileContext(nc) as tc, nc.sync.register() as cnt_reg:
        with (
            tc.tile_pool(name="cp", bufs=1) as cpool,
            tc.tile_pool(name="wp", bufs=1) as wpool,
            tc.tile_pool(name="xp", bufs=1) as xpool,
            tc.tile_pool(name="yp", bufs=1) as ypool,
            tc.tile_pool(name="ps", bufs=8, space="PSUM") as pspool,
        ):
            xch = {}

            def xdma(u, cond=None):
                t = xpool.tile([CIN, UNIT], F16, tag=f"x{u}", name=f"xch{u}")
                if cond is None:
                    nc.sync.dma_start(t[:], x[u])
                else:
                    nc.sync.dma_start(t[:], x[u], cond=cond, cond_hint=True)
                xch[u] = t

            # first real unit's data leads the stream; wT/cnt are tiny and
            # needed only ~4us later
            xdma(N_COND)
            w_t = wpool.tile([CIN, COUT], F16)
            nc.sync.dma_start(w_t[:], wT[:])
            cnt_t = cpool.tile([1, 1], I32)
            nc.sync.dma_start(cnt_t[:], cnt[:])
            for u in range(N_COND + 1, U_MAX):
                xdma(u)

            # per-core unit count -> predicates for the leading units
            nc.sync.reg_load(cnt_reg, cnt_t[0:1, 0:1])
            cnt_v = nc.sync.snap(cnt_reg, min_val=U_MIN, max_val=U_MAX)
            conds = {u: cnt_v > U_MAX - 1 - u for u in range(N_COND)}
            for u in range(N_COND):
                xdma(u, cond=conds[u])

            ych = [
                ypool.tile([COUT, UNIT], F16, tag=f"y{u}", name=f"ych{u}")
                for u in range(U_MAX)
            ]

            for u in u_order:
                for j in range(MM_PER_U):
                    ps = pspool.tile(
                        [COUT, MMN], F32, tag="ps", name=f"ps{u}_{j}"
                    )
                    # skipped units just matmul stale SBUF; their output
                    # DMA is predicated off so nothing escapes
                    nc.tensor.matmul(
                        ps[:],
                        w_t[:],
                        xch[u][:, MMN * j:MMN * (j + 1)],
                        start=True,
                        stop=True,
                    )
                    dst = ych[u][:, MMN * j:MMN * (j + 1)]
                    # split evacuation DVE/ACT so neither lags the DMA cadence
                    if j % 2 == 0:
                        nc.vector.tensor_copy(dst, ps[:])
                    else:
                        nc.scalar.copy(dst, ps[:])
                if u >= N_COND:
                    nc.sync.dma_start(y[u], ych[u][:])
                else:
                    nc.sync.dma_start(
                        y[u], ych[u][:], cond=conds[u], cond_hint=True
                    )
    nc.compile()
    return nc


_NC_CACHE = {}


def _get_nc():
    if "nc" not in _NC_CACHE:
        _NC_CACHE["nc"] = _build_nc()
    return _NC_CACHE["nc"]


def _prep_in_maps(x, alphas):
    """host prep: spatial flip + fp16 cast (layout/precision only, no math)
    then carve the 64 (batch, chunk) units into per-core contiguous spans"""
    x16 = np.asarray(x).astype(np.float16)
    xf = x16[:, :, _FLIP[:, None], _FLIP[None, :]].reshape(B, CIN, HW)
    wT = np.ascontiguousarray(
        np.asarray(alphas, dtype=np.float32).T
    ).astype(np.float16)
    in_maps = []
    for c in range(N_CORES):
        xarr = np.zeros((U_MAX, CIN, UNIT), dtype=np.float16)
        for i in range(COUNTS[c]):
            g = STARTS[c] + i
            b, k = divmod(g, KPB)
            xarr[U_MAX - COUNTS[c] + i] = xf[b, :, UNIT * k:UNIT * (k + 1)]
        in_maps.append(
            {
                "x": xarr,
                "wT": wT,
                "cnt": np.array([[COUNTS[c]]], dtype=np.int32),
            }
        )
    return in_maps


def kernel(x, alphas, betas=None, **_unused):
    nc = _get_nc()
    in_maps = _prep_in_maps(x, alphas)
    res = run_bass_kernel_spmd(nc, in_maps, core_ids=list(range(N_CORES)))
    out = np.empty((B, COUT, HW), dtype=np.float32)
    for c in range(N_CORES):
        yarr = np.asarray(res.results[c]["y"], dtype=np.float32)
        for i in range(COUNTS[c]):
            g = STARTS[c] + i
            b, k = divmod(g, KPB)
            out[b, :, UNIT * k:UNIT * (k + 1)] = yarr[U_MAX - COUNTS[c] + i]
    return out.reshape(B, COUT, H, W)


# revision 22
# speedup vs baseline: 2.6621x; 1.1342x over previous
"""Trainium2 Bass kernel for nn_KKLayer (spectral channel-mix layer).

Math identity: the reference computes
    y = Re(IFFT2((A + iB) . conj(FFT2(x))))            (channel mix in freq domain)
Since channel mixing commutes with the spatial FFT and, for real x,
IFFT2(conj(FFT2(x))) is x spatially "negated" (h -> (-h) mod H, w -> (-w) mod W),
the whole layer collapses to
    y[b,o,h,w] = sum_i A[o,i] * x[b,i,(H-h)%H,(W-w)%W]
(betas drop out of the real part entirely).

Kernel: the work is split into 64 independent units (batch x column-chunk,
each 512KB in + 512KB out in fp16) spread over the 8 cores. The spatial flip
is a pure layout permutation, applied on the host while converting x to fp16
(rel-err gate is 2e-2; measured ~5e-4), so every device access is contiguous
and HBM traffic is halved vs fp32.

Core 0 demonstrably runs its DMA engines slower than the other cores (one
engine serves host/runtime traffic and lags ~20%), so the unit counts are
load-balanced: core 0 takes 6 units, the fastest-measured cores take 9, the
rest 8 - equalizing per-core finish times. The per-core count is passed as a
scalar input and the tail unit DMAs are predicated on it (cond=), so a single
SPMD program serves all cores.

Per unit: 1 input DMA, 4 matmuls [K=128, M=128, N=512] fp16 -> fp32 PSUM,
4 PSUM->SBUF copies casting to fp16 (2 on DVE + 2 on ACT), 1 output DMA.
"""

import numpy as np

import concourse.bass as bass
import concourse.bacc as bacc
import concourse.mybir as mybir
from concourse import tile
from concourse.bass_utils import run_bass_kernel_spmd

B, CIN, COUT, H, W = 8, 128, 128, 128, 128
HW = H * W              # 16384
N_CORES = 8
UNIT = 2048             # columns per unit (4KB/partition in fp16)
KPB = HW // UNIT        # 8 column chunks per batch
N_UNITS = B * KPB       # 64 total units
MMN = 512               # matmul free dim (one PSUM bank of fp32)
MM_PER_U = UNIT // MMN  # 4

# units per core: core 0 is the host-facing (slow-DMA) core; cores 1,3
# measured fastest. sum == N_UNITS.
COUNTS = (6, 9, 8, 9, 8, 8, 8, 8)
assert sum(COUNTS) == N_UNITS
U_MAX = max(COUNTS)     # SPMD program sized for the largest share
U_MIN = min(COUNTS)     # units below this run unconditionally
STARTS = tuple(int(s) for s in np.cumsum((0,) + COUNTS[:-1]))

F16 = mybir.dt.float16
F32 = mybir.dt.float32
I32 = mybir.dt.int32

# dest row h reads source row (H - h) % H
_FLIP = (-np.arange(H)) % H


def _build_nc():
    nc = bacc.Bacc(None, target_bir_lowering=False)
    x = nc.dram_tensor("x", [U_MAX, CIN, UNIT], F16, kind="ExternalInput")
    wT = nc.dram_tensor("wT", [CIN, COUT], F16, kind="ExternalInput")
    cnt = nc.dram_tensor("cnt", [1, 1], I32, kind="ExternalInput")
    y = nc.dram_tensor("y", [U_MAX, COUT, UNIT], F16, kind="ExternalOutput")

    # units [U_MAX-cnt, U_MAX) are active on a core; units below that are
    # skipped. The unconditional units (>= N_COND) are issued/computed
    # FIRST so the conditional (possibly garbage) tail never delays a
    # core's real work, and every core's compute follows arrival order.
    N_COND = U_MAX - U_MIN
    u_order = list(range(N_COND, U_MAX)) + list(range(N_COND))

    with tile.TileContext(nc) as tc, nc.sync.register() as cnt_reg:
        with (
            tc.tile_pool(name="cp", bufs=1) as cpool,
            tc.tile_pool(name="wp", bufs=1) as wpool,
            tc.tile_pool(name="xp", bufs=1) as xpool,
            tc.tile_pool(name="yp", bufs=1) as ypool,
            tc.tile_pool(name="ps", bufs=8, space="PSUM") as pspool,
        ):
            xch = {}

            def xdma(u, cond=None):
                t = xpool.tile([CIN, UNIT], F16, tag=f"x{u}", name=f"xch{u}")
                if cond is None:
                    nc.sync.dma_start(t[:], x[u])
                else:
                    nc.sync.dma_start(t[:], x[u], cond=cond, cond_hint=True)
                xch[u] = t

            # first real unit's data leads the stream; wT/cnt are tiny and
            # needed only ~4us later. The first unit is split in two
            # half-partition DMAs: slower per byte, but every engine
            # (notably the late-waking one serving partitions 92-95/124-127)
            # gets descriptors within the first generation pass.
            t0 = xpool.tile(
                [CIN, UNIT], F16, tag=f"x{N_COND}", name=f"xch{N_COND}"
            )
            nc.sync.dma_start(t0[64:128], x[N_COND][64:128])
            nc.sync.dma_start(t0[0:64], x[N_COND][0:64])
            xch[N_COND] = t0
            w_t = wpool.tile([CIN, COUT], F16)
            nc.sync.dma_start(w_t[:], wT[:])
            cnt_t = cpool.tile([1, 1], I32)
            nc.sync.dma_start(cnt_t[:], cnt[:])
            for u in range(N_COND + 1, U_MAX):
                xdma(u)
            assert len(xch) == U_MIN

            # per-core unit count -> predicates for the leading units
            nc.sync.reg_load(cnt_reg, cnt_t[0:1, 0:1])
            cnt_v = nc.sync.snap(cnt_reg, min_val=U_MIN, max_val=U_MAX)
            conds = {u: cnt_v > U_MAX - 1 - u for u in range(N_COND)}
            for u in range(N_COND):
                xdma(u, cond=conds[u])

            ych = [
                ypool.tile([COUT, UNIT], F16, tag=f"y{u}", name=f"ych{u}")
                for u in range(U_MAX)
            ]

            for u in u_order:
                for j in range(MM_PER_U):
                    ps = pspool.tile(
                        [COUT, MMN], F32, tag="ps", name=f"ps{u}_{j}"
                    )
                    # skipped units just matmul stale SBUF; their output
                    # DMA is predicated off so nothing escapes
                    nc.tensor.matmul(
                        ps[:],
                        w_t[:],
                        xch[u][:, MMN * j:MMN * (j + 1)],
                        start=True,
                        stop=True,
                    )
                    dst = ych[u][:, MMN * j:MMN * (j + 1)]
                    # split evacuation DVE/ACT so neither lags the DMA cadence
                    if j % 2 == 0:
                        nc.vector.tensor_copy(dst, ps[:])
                    else:
                        nc.scalar.copy(dst, ps[:])
                if u >= N_COND:
                    nc.sync.dma_start(y[u], ych[u][:])
                else:
                    nc.sync.dma_start(
                        y[u], ych[u][:], cond=conds[u], cond_hint=True
                    )
    nc.compile()
    return nc


_NC_CACHE = {}


def _get_nc():
    if "nc" not in _NC_CACHE:
        _NC_CACHE["nc"] = _build_nc()
    return _NC_CACHE["nc"]


def _prep_in_maps(x, alphas):
    """host prep: spatial flip + fp16 cast (layout/precision only, no math)
    then carve the 64 (batch, chunk) units into per-core contiguous spans"""
    x16 = np.asarray(x).astype(np.float16)
    xf = x16[:, :, _FLIP[:, None], _FLIP[None, :]].reshape(B, CIN, HW)
    wT = np.ascontiguousarray(
        np.asarray(alphas, dtype=np.float32).T
    ).astype(np.float16)
    in_maps = []
    for c in range(N_CORES):
        xarr = np.zeros((U_MAX, CIN, UNIT), dtype=np.float16)
        for i in range(COUNTS[c]):
            g = STARTS[c] + i
            b, k = divmod(g, KPB)
            xarr[U_MAX - COUNTS[c] + i] = xf[b, :, UNIT * k:UNIT * (k + 1)]
        in_maps.append(
            {
                "x": xarr,
                "wT": wT,
                "cnt": np.array([[COUNTS[c]]], dtype=np.int32),
            }
        )
    return in_maps


def kernel(x, alphas, betas=None, **_unused):
    nc = _get_nc()
    in_maps = _prep_in_maps(x, alphas)
    res = run_bass_kernel_spmd(nc, in_maps, core_ids=list(range(N_CORES)))
    out = np.empty((B, COUT, HW), dtype=np.float32)
    for c in range(N_CORES):
        yarr = np.asarray(res.results[c]["y"], dtype=np.float32)
        for i in range(COUNTS[c]):
            g = STARTS[c] + i
            b, k = divmod(g, KPB)
            out[b, :, UNIT * k:UNIT * (k + 1)] = yarr[U_MAX - COUNTS[c] + i]
    return out.reshape(B, COUT, H, W)
